# revision 3
# baseline (speedup 1.0000x reference)
"""Trainium2 Bass kernel for a 6-layer transformer decoder (D=1024, H=16, FF=4096).

Sharding: data-parallel over batch — each of the 8 NeuronCores processes one
batch element end-to-end (no collectives).

On-chip layout: activations are kept feature-major ("xT": [D, T] stored as
[128 partitions, D/128, T]) so every linear layer is a natural
`out = W.T @ xT` PE matmul (lhsT = natural-layout weight chunks), layernorm
statistics are computed with ones-vector matmuls on the PE, and per-token
scalars are broadcast across partitions with K=1 outer-product matmuls.
Attention scores are computed transposed (S^T = K Q^T per head) so softmax
normalization lands on the free axis and attn@V consumes the probabilities
directly, with a ones-column appended to V to produce the softmax denominator
in the same accumulation.

Layernorm is algebraically commuted past the following projections so the PE
never waits for the normalization statistics: the projections run on the
*pre-norm* activations, the mean-centering enters each matmul accumulation as
a rank-1 seed (host-precomputed weight column sums x (-mean), one K=1 matmul
appended to each accumulation group), and the 1/std scale is applied during
psum evacuation (for the FFN it commutes through ReLU and the second matmul
and is applied at the FFN2 evacuation).

Matmul operands are bf16 (fp32 PSUM accumulation); the residual stream and
softmax/layernorm statistics stay fp32.
"""
import numpy as np
import ml_dtypes

BF16 = ml_dtypes.bfloat16

L, D, H, FF = 6, 1024, 16, 4096
DH = D // H          # 64
T = 512              # sequence length (both q and kv)
P = 128
JD = D // P          # 8 feature chunks
JF = FF // P         # 32 ff chunks
JH = JF // 2         # ff chunks per half
TC = T // P          # 4 token chunks
NCORES = 8
EPS = 1e-12
VW = DH + 1          # v columns per head incl. ones column (65)
WS = 4 * D + FF      # wsum row length (saq | sak | caq | sav | w1)
OFF_SAQ, OFF_SAK, OFF_CAQ, OFF_SAV, OFF_W1 = 0, D, 2 * D, 3 * D, 4 * D

_CACHE = {}
TRACE_TMPDIR = None  # test harness may set this to capture a profile


def _build_program(n_layers=L, loop_n=1):
    import concourse.bass as bass
    import concourse.mybir as mybir
    import concourse.tile as tile
    from concourse import bacc

    f32 = mybir.dt.float32
    bf16 = mybir.dt.bfloat16
    AF = mybir.ActivationFunctionType
    OP = mybir.AluOpType

    class _OneActTableBacc(bacc.Bacc):
        """This kernel only uses exp/ln/relu/square/copy, all of which live in
        the `natural_log_exp_and_others` activation-table set.  The stock
        first-fit table choice pairs `ln` with a set that lacks `exp`, causing
        a table reload on every layernorm; presenting only the one covering
        set (other slots empty, list order preserved so set ids still match
        act_info.json) collapses all reloads into a single load."""

        def insert_act_table_loads(self):
            from concourse.hw_specs import get_activation_tables
            import bass_rust as _bass_rust
            has_activation = any(
                isinstance(i, mybir.InstActivation)
                for b in self.main_func.blocks
                for i in b.instructions
            )
            if not has_activation:
                return
            tables = list(get_activation_tables(self.m.arch).items())
            keep = "natural_log_exp_and_others"
            assert any(nm == keep for nm, _ in tables)
            tables = [(nm, (fns if nm == keep else set()))
                      for nm, fns in tables]
            _bass_rust.insert_act_table_loads(self, tables)

    nc = _OneActTableBacc(
        "TRN2",
        target_bir_lowering=False,
        debug=False,
        enable_asserts=False,
        num_devices=NCORES,
    )

    # ---- DRAM tensors (per-core shapes) ----
    # inner dims flattened so each DMA is one contiguous run per partition
    xT_d = nc.dram_tensor("xT", [P, JD * T], f32, kind="ExternalInput").ap()
    xTb_d = nc.dram_tensor("xTb", [P, JD * T], bf16, kind="ExternalInput").ap()
    encT_d = nc.dram_tensor("encT", [P, JD * T], bf16, kind="ExternalInput").ap()
    mask_d = nc.dram_tensor("mask01", [P, TC * T], bf16, kind="ExternalInput").ap()
    wnames = ["saq", "sak", "sav", "sao", "caq", "cak", "cav", "cao"]
    wd = {
        n: nc.dram_tensor(n, [L, 2, P, JD * (D // 2)], bf16,
                          kind="ExternalInput").ap()
        for n in wnames
    }
    w1_d = nc.dram_tensor("w1", [L, JF, P, JD * P], bf16, kind="ExternalInput").ap()
    w2_d = nc.dram_tensor("w2", [L, JD, P, JF * P], bf16, kind="ExternalInput").ap()
    ws_d = nc.dram_tensor("wsums", [L, 1, 4 * D], bf16, kind="ExternalInput").ap()
    wsf_d = nc.dram_tensor("wsumf", [L, 1, FF], bf16, kind="ExternalInput").ap()
    out_d = nc.dram_tensor("outT", [P, JD * T], f32, kind="ExternalOutput").ap()

    with tile.TileContext(nc) as tc:
        _emit(tc, nc, bass, mybir, tile, f32, bf16, AF, OP, xT_d, xTb_d,
              encT_d, mask_d, wd, w1_d, w2_d, ws_d, wsf_d, out_d, n_layers,
              loop_n)
    nc.compile()
    return nc


def _emit(tc, nc, bass, mybir, tile, f32, bf16, AF, OP, xT_d, xTb_d,
          encT_d, mask_d, wd, w1_d, w2_d, ws_d, wsf_d, out_d, n_layers=L,
          loop_n=1):
    from contextlib import ExitStack
    ctx = ExitStack()
    with ctx:
        persist = ctx.enter_context(tc.tile_pool(name="persist", bufs=1))
        wpool = ctx.enter_context(tc.tile_pool(name="wpool", bufs=4))
        w1pool = ctx.enter_context(tc.tile_pool(name="w1pool", bufs=4))
        w2pool = ctx.enter_context(tc.tile_pool(name="w2pool", bufs=2))
        wspool = ctx.enter_context(tc.tile_pool(name="wspool", bufs=1))
        apool = ctx.enter_context(tc.tile_pool(name="apool", bufs=1))
        lnpool = ctx.enter_context(tc.tile_pool(name="lnpool", bufs=1))
        spool = ctx.enter_context(tc.tile_pool(name="spool", bufs=2))
        abpool = ctx.enter_context(tc.tile_pool(name="abpool", bufs=4))
        probpool = ctx.enter_context(tc.tile_pool(name="probpool", bufs=2))
        rows = ctx.enter_context(tc.tile_pool(name="rows", bufs=2))
        rows1 = ctx.enter_context(tc.tile_pool(name="rows1", bufs=1))
        hpool = ctx.enter_context(tc.tile_pool(name="hpool", bufs=1))
        # PSUM pools: 8 banks total
        pproj = ctx.enter_context(tc.tile_pool(name="pproj", bufs=2, space="PSUM"))
        psc = ctx.enter_context(tc.tile_pool(name="psc", bufs=2, space="PSUM"))
        pao = ctx.enter_context(tc.tile_pool(name="pao", bufs=2, space="PSUM"))

        # ---- persistent tiles ----
        xT_f = persist.tile([P, JD * T], f32, tag="xT")     # residual stream
        encTb_f = persist.tile([P, JD * T], bf16, tag="encTb")
        mask_f = persist.tile([P, TC * T], bf16, tag="mask01")
        xT = xT_f.rearrange("p (j t) -> p j t", t=T)
        encTb = encTb_f.rearrange("p (j t) -> p j t", t=T)
        mask01 = mask_f.rearrange("p (c t) -> p c t", t=T)
        # layer-0's bf16 input x lives in the (single) yb slot until the
        # first layernorm claims it
        xTb_t = lnpool.tile([P, JD, T], bf16, tag="yb")
        xTb = xTb_t[:]
        ones_b = persist.tile([P, P], bf16, tag="ones_b")
        ones_f = persist.tile([1, P], f32, tag="ones_f")
        eps_c = persist.tile([1, 1], f32, tag="eps_c")
        y_f = persist.tile([P, JD * T], f32, tag="y")
        y = y_f.rearrange("p (j t) -> p j t", t=T)

        # xTb feeds the very first matmuls — keep it alone on the sync queue
        # so the first weight loads queue right behind it; the rest goes via
        # the gpsimd DMA queue in parallel
        nc.sync.dma_start(xTb_t[:], xTb_d.rearrange("p (j t) -> p j t", t=T))
        nc.gpsimd.dma_start(xT_f[:], xT_d[:])
        nc.gpsimd.dma_start(encTb_f[:], encT_d[:])
        nc.gpsimd.dma_start(mask_f[:], mask_d[:])
        nc.vector.memset(eps_c[:], EPS)
        nc.vector.memset(ones_b[:], 1.0)
        nc.vector.memset(ones_f[:], 1.0)

        def load_wmat(name, l, half):
            """Half of a [Din, Dout] mat: output columns half*512..+512,
            viewed [P, JD, 512]."""
            wt = wpool.tile([P, JD * (D // 2)], bf16, tag="wmat")
            nc.sync.dma_start(wt[:], wd[name][l, half])
            return wt.rearrange("p (k n) -> p k n", n=D // 2)

        # ---------------- layernorm (commuted) ----------------
        class LN:
            """Stats accumulate while the producing projection evacuates;
            centering/scale are applied by the *consuming* projections."""

            def __init__(self, need_cols=False):
                self.need_cols = need_cols

            def start(self):
                self.yb = lnpool.tile([P, JD, T], bf16, tag="yb")
                self.pm = psc.tile([1, T], f32, tag="sc")
                self.pss = psc.tile([1, T], f32, tag="sc")
                self.bcasts_done = False

            def chunk(self, j):
                """Call after y[:, j, :] is written."""
                nc.scalar.copy(out=self.yb[:, j, :], in_=y[:, j, :])
                sq = spool.tile([P, T], bf16, tag="ysqc")
                nc.scalar.activation(out=sq[:], in_=y[:, j, :], func=AF.Square)
                nc.tensor.matmul(self.pm[:], lhsT=ones_b[:, 0:1],
                                 rhs=self.yb[:, j, :],
                                 start=(j == 0), stop=(j == JD - 1))
                nc.tensor.matmul(self.pss[:], lhsT=ones_b[:, 0:1], rhs=sq[:],
                                 start=(j == 0), stop=(j == JD - 1))

            def finish_rows(self):
                # -mean in bf16: doubles as the rank-1 seed operand, and the
                # bf16 rounding is negligible against the bf16 matmul inputs
                m_neg = rows.tile([1, T], bf16, tag="mrow")
                with nc.allow_low_precision(reason="mean in bf16"):
                    nc.vector.tensor_scalar_mul(m_neg[:], self.pm[:], -1.0 / D)
                m2 = rows1.tile([1, T], f32, tag="lr1")
                nc.vector.tensor_tensor(out=m2[:], in0=m_neg[:], in1=m_neg[:],
                                        op=OP.mult)
                var = rows1.tile([1, T], f32, tag="lr2")
                nc.vector.scalar_tensor_tensor(
                    out=var[:], in0=self.pss[:], scalar=1.0 / D, in1=m2[:],
                    op0=OP.mult, op1=OP.subtract)
                lnv = rows1.tile([1, T], f32, tag="lr1")
                nc.scalar.activation(out=lnv[:], in_=var[:], func=AF.Ln,
                                     bias=eps_c[:])
                rstd = rows1.tile([1, T], f32, tag="rstd")
                nc.scalar.activation(out=rstd[:], in_=lnv[:], func=AF.Exp,
                                     scale=-0.5)
                c_row = rows1.tile([1, T], f32, tag="crow")
                nc.vector.tensor_tensor(out=c_row[:], in0=m_neg[:],
                                        in1=rstd[:], op=OP.mult)
                self.mnegb, self.rstd, self.c_row = m_neg, rstd, c_row
                if self.need_cols:
                    # token-indexed columns of rstd and 1/rstd (for scaling
                    # the next self-attention's probabilities / V ones col):
                    # tiny K=1,N=1 matmuls transpose the rows
                    std = rows1.tile([1, T], f32, tag="lr2")
                    nc.scalar.activation(out=std[:], in_=lnv[:], func=AF.Exp,
                                         scale=0.5)
                    pcol = psc.tile([P, 2 * TC], f32, tag="sc")
                    for c in range(TC):
                        nc.tensor.matmul(
                            pcol[:, c:c + 1],
                            lhsT=self.rstd[0:1, c * P:(c + 1) * P],
                            rhs=ones_f[0:1, 0:1],
                            start=True, stop=True)
                        nc.tensor.matmul(
                            pcol[:, TC + c:TC + c + 1],
                            lhsT=std[0:1, c * P:(c + 1) * P],
                            rhs=ones_f[0:1, 0:1],
                            start=True, stop=True)
                    cols = rows.tile([P, 2 * TC], f32, tag="cols")
                    nc.vector.tensor_copy(out=cols[:], in_=pcol[:])
                    self.a_col = cols[:, 0:TC]
                    self.inva_col = cols[:, TC:2 * TC]

            def emit_bcasts(self):
                if self.bcasts_done:
                    return
                self.bcasts_done = True
                pa = psc.tile([P, T], f32, tag="sc")
                nc.tensor.matmul(pa[:], lhsT=ones_f[:, :], rhs=self.rstd[:],
                                 start=True, stop=True)
                self.a_sb = abpool.tile([P, T], f32, tag="ab")
                nc.scalar.copy(out=self.a_sb[:], in_=pa[:])
                pc = psc.tile([P, T], f32, tag="sc")
                nc.tensor.matmul(pc[:], lhsT=ones_f[:, :], rhs=self.c_row[:],
                                 start=True, stop=True)
                self.c_sb = abpool.tile([P, T], f32, tag="ab")
                nc.scalar.copy(out=self.c_sb[:], in_=pc[:])

            def emit_tail(self):
                """Materialize the normalized x (fp32, for the residual adds),
                chunked so it pipelines with surrounding DVE work."""
                self.emit_bcasts()
                for j in range(JD):
                    nc.vector.tensor_tensor(out=y[:, j, :], in0=y[:, j, :],
                                            in1=self.a_sb[:], op=OP.mult)
                    nc.vector.tensor_tensor(out=xT[:, j, :], in0=y[:, j, :],
                                            in1=self.c_sb[:], op=OP.add)

        def proj_seeded(w_sb, ln, ws_sb, ws_off, dst):
            """Feature-major projection reading pre-norm yb, centered by the
            wsum x (-mean) rank-1 seed, scaled by rstd at evacuation."""
            for j in range(JD):
                ps = pproj.tile([P, T], f32, tag="proj")
                for kc in range(JD):
                    nc.tensor.matmul(
                        ps[:],
                        lhsT=w_sb[:, kc, j * P:(j + 1) * P],
                        rhs=ln.yb[:, kc, :],
                        start=(kc == 0), stop=False)
                nc.tensor.matmul(
                    ps[:],
                    lhsT=ws_sb[0:1, ws_off + j * P:ws_off + (j + 1) * P],
                    rhs=ln.mnegb[:],
                    start=False, stop=True)
                ln.emit_bcasts()
                nc.vector.tensor_tensor(out=dst[:, j, :], in0=ps[:],
                                        in1=ln.a_sb[:], op=OP.mult)

        def proj_plain(w_sb, src_b, dst):
            for j in range(JD):
                ps = pproj.tile([P, T], f32, tag="proj")
                for kc in range(JD):
                    nc.tensor.matmul(
                        ps[:],
                        lhsT=w_sb[:, kc, j * P:(j + 1) * P],
                        rhs=src_b[:, kc, :],
                        start=(kc == 0), stop=(kc == JD - 1))
                nc.scalar.copy(out=dst[:, j, :], in_=ps[:])

        def attn_block(wnames, l, ws_sb, ln_src, kv_b, mask, ln_next,
                       tail_ln=None):
            """One attention block with interleaved emission: q/k/v projection
            groups are threaded between the per-head softmax chains so the PE
            stays dense while exp/normalize resolve on ACT/DVE.

            ln_src: LN whose (pre-norm) activations feed q [and k,v for
            self-attention]; None -> plain reads of xTb (layer-0 self-attn).
            kv_b: bf16 feature-major source for k/v when they are NOT fed by
            ln_src (cross-attention: encTb; layer-0 self-attn: xTb).
            tail_ln: LN whose fp32 x tail must materialize before the output
            projection's residual add (emitted late, after the head chains).
            """
            wq_name, wk_name, wv_name, wo_name = wnames
            self_attn = wv_name == "sav"
            seeded = ln_src is not None
            sa_seeded = seeded and self_attn

            qTb = apool.tile([P, JD, T], bf16, tag="qTb")
            kTb = apool.tile([P, JD, T], bf16, tag="kTb")
            vtok = apool.tile([P, TC, H * VW], bf16, tag="vtok")
            vt4 = vtok.rearrange("p tc (h w) -> p tc h w", w=VW)
            wtiles = {}

            def wmat(name, half):
                if (name, half) not in wtiles:
                    wtiles[(name, half)] = load_wmat(name, l, half)
                return wtiles[(name, half)]

            def qk_group(j):
                half, jj = j // 4, j % 4
                for nm, dst, off in ((wq_name, qTb, OFF_SAQ if self_attn
                                      else OFF_CAQ),
                                     (wk_name, kTb, OFF_SAK)):
                    w_sb = wmat(nm, half)
                    if seeded and (self_attn or dst is qTb):
                        ln = ln_src
                        ps = pproj.tile([P, T], f32, tag="proj")
                        for kc in range(JD):
                            nc.tensor.matmul(
                                ps[:], lhsT=w_sb[:, kc, jj * P:(jj + 1) * P],
                                rhs=ln.yb[:, kc, :],
                                start=(kc == 0), stop=False)
                        nc.tensor.matmul(
                            ps[:],
                            lhsT=ws_sb[0:1, off + j * P:off + (j + 1) * P],
                            rhs=ln.mnegb[:], start=False, stop=True)
                        ln.emit_bcasts()
                        nc.vector.tensor_tensor(out=dst[:, j, :], in0=ps[:],
                                                in1=ln.a_sb[:], op=OP.mult)
                    else:
                        ps = pproj.tile([P, T], f32, tag="proj")
                        for kc in range(JD):
                            nc.tensor.matmul(
                                ps[:], lhsT=w_sb[:, kc, jj * P:(jj + 1) * P],
                                rhs=kv_b[:, kc, :],
                                start=(kc == 0), stop=(kc == JD - 1))
                        nc.scalar.copy(out=dst[:, j, :], in_=ps[:])

            def v_group(tc4, jh):
                wv = wmat(wv_name, jh)
                ps = pproj.tile([P, T], f32, tag="proj")
                if sa_seeded:
                    for kc in range(JD):
                        nc.tensor.matmul(
                            ps[:],
                            lhsT=ln_src.yb[:, kc, tc4 * P:(tc4 + 1) * P],
                            rhs=wv[:, kc, :],
                            start=(kc == 0), stop=False)
                    # centering seed: outer(-mean[tc4 chunk], wsum_v[jh half])
                    nc.tensor.matmul(
                        ps[:],
                        lhsT=ln_src.mnegb[0:1, tc4 * P:(tc4 + 1) * P],
                        rhs=ws_sb[0:1, OFF_SAV + jh * 512:
                                  OFF_SAV + (jh + 1) * 512],
                        start=False, stop=True)
                else:
                    for kc in range(JD):
                        nc.tensor.matmul(
                            ps[:],
                            lhsT=kv_b[:, kc, tc4 * P:(tc4 + 1) * P],
                            rhs=wv[:, kc, :],
                            start=(kc == 0), stop=(kc == JD - 1))
                nc.scalar.copy(
                    out=vt4[:, tc4, jh * 8:(jh + 1) * 8, 0:DH],
                    in_=ps.rearrange("p (h d) -> p h d", d=DH))

            # softmax denominator column of V: 1/rstd when the probabilities
            # carry the rstd scale, else plain ones
            if sa_seeded:
                nc.vector.tensor_copy(
                    out=vt4[:, :, :, DH:DH + 1],
                    in_=ln_src.inva_col[:, :, None, None].to_broadcast(
                        [P, TC, H, 1]))
            else:
                nc.vector.memset(vt4[:, :, :, DH:DH + 1], 1.0)

            def head_pair(hp):
                """Heads 2hp (partitions 0:64) and 2hp+1 (64:128) share the
                q/k chunk jh=hp; their score matmuls land in one two-bank
                psum tile via concurrent PE row-groups, so exp and
                mask/scale run once per ktc at double width."""
                jh = hp
                # 2-deep ktc ring of pair-probs (both heads side by side)
                probs = probpool.tile([P, 2, 2, T], bf16, tag="probs")
                po = [pao.tile([P, T], f32, tag="ao", name=f"po{i}")
                      for i in range(2)]
                for ktc in range(TC):
                    ss = psc.tile([P, 2, T], f32, tag="sc")
                    for hh in range(2):
                        ph = hh * 64
                        nc.tensor.matmul(
                            ss[:, hh, :],
                            lhsT=kTb[ph:ph + 64, jh, ktc * P:(ktc + 1) * P],
                            rhs=qTb[ph:ph + 64, jh, :],
                            start=True, stop=True)
                    pr = probs[:, ktc % 2, :, :]
                    nc.scalar.activation(
                        out=pr, in_=ss[:], func=AF.Exp, scale=0.125)
                    if sa_seeded:
                        # fold the V-side rstd (token-indexed, so per-
                        # partition here) and the mask into one pass
                        if mask is not None:
                            nc.vector.scalar_tensor_tensor(
                                out=pr, in0=pr,
                                scalar=ln_src.a_col[:, ktc:ktc + 1],
                                in1=mask[:, ktc, None, :].to_broadcast(
                                    [P, 2, T]),
                                op0=OP.mult, op1=OP.mult)
                        else:
                            nc.vector.tensor_scalar_mul(
                                pr, pr, ln_src.a_col[:, ktc:ktc + 1])
                    elif mask is not None:
                        nc.vector.tensor_tensor(
                            out=pr, in0=pr,
                            in1=mask[:, ktc, None, :].to_broadcast([P, 2, T]),
                            op=OP.mult)
                    for hh in range(2):
                        nc.tensor.matmul(
                            po[hh][0:VW, :],
                            lhsT=vt4[:, ktc, 2 * hp + hh, :],
                            rhs=probs[:, ktc % 2, hh, :],
                            start=(ktc == 0), stop=(ktc == TC - 1))
                for hh in range(2):
                    ph = hh * 64
                    r_row = rows.tile([1, T], bf16, tag="rrow")
                    with nc.allow_low_precision(reason="softmax denom bf16"):
                        nc.vector.reciprocal(out=r_row[:],
                                             in_=po[hh][DH:DH + 1, :])
                    pb = psc.tile([P, 2, T], f32, tag="sc")
                    nc.tensor.matmul(
                        pb[0:DH, 0, :], lhsT=ones_b[0:1, 0:DH], rhs=r_row[:],
                        start=True, stop=True)
                    bb = spool.tile([DH, T], bf16, tag="bcast_sb")
                    nc.vector.tensor_copy(out=bb[:], in_=pb[0:DH, 0, :])
                    nc.vector.tensor_tensor(
                        out=attnTb[ph:ph + 64, jh, :], in0=po[hh][0:DH, :],
                        in1=bb[:], op=OP.mult)

            attnTb = apool.tile([P, JD, T], bf16, tag="attnTb")
            # interleaved emission: enough q/k/v groups up front for the
            # first head pairs, the rest threaded between head chains
            qk_group(0)
            qk_group(1)
            for tc4 in range(TC):
                v_group(tc4, 0)
            extra = {0: [("qk", 2)], 1: [("qk", 3), ("v", 0), ("v", 1)],
                     2: [("qk", 4), ("v", 2), ("v", 3)], 3: [("qk", 5)],
                     4: [("qk", 6)], 5: [("qk", 7)]}
            for hp in range(JD):
                head_pair(hp)
                for kind, idx in extra.get(hp, []):
                    if kind == "qk":
                        qk_group(idx)
                    else:
                        v_group(idx, 1)
            if tail_ln is not None:
                tail_ln.emit_tail()
            ln_next.start()
            for hp in range(JD):
                # output projection (+ residual + LN stats chunk)
                wo = wmat(wo_name, hp // 4)
                ps = pproj.tile([P, T], f32, tag="proj")
                for kc in range(JD):
                    nc.tensor.matmul(
                        ps[:],
                        lhsT=wo[:, kc, (hp % 4) * P:(hp % 4 + 1) * P],
                        rhs=attnTb[:, kc, :],
                        start=(kc == 0), stop=(kc == JD - 1))
                nc.vector.tensor_tensor(out=y[:, hp, :], in0=ps[:],
                                        in1=xT[:, hp, :], op=OP.add)
                ln_next.chunk(hp)
            ln_next.finish_rows()

        def layers_body():
            ln_prev = None  # LN3 of the previous layer (None for layer 0)
            for l in range(n_layers):
                ws_sb = wspool.tile([1, 4 * D], bf16, tag="wsums")
                nc.sync.dma_start(ws_sb[:], ws_d[l])

                # --- self-attention block ---
                # (in the For_i timing build the layer-0 input tile's slot is
                # recycled by later layernorms, so point it at a persistent
                # tile instead; timing-equivalent, numerics unused there)
                x0 = encTb if loop_n > 1 else xTb
                ln1 = LN()
                attn_block(("saq", "sak", "sav", "sao"), l, ws_sb, ln_prev,
                           x0, mask01, ln1, tail_ln=ln_prev)

                # --- cross-attention block ---
                ln2 = LN()
                attn_block(("caq", "cak", "cav", "cao"), l, ws_sb, ln1,
                           encTb, None, ln2, tail_ln=ln1)

                # --- FFN ---
                wsf_sb = wspool.tile([1, FF], bf16, tag="wsumf")
                nc.sync.dma_start(wsf_sb[:], wsf_d[l])
                ln3 = LN(need_cols=True)
                ln3.start()
                for half in range(2):
                    hT = hpool.tile([P, JH, T], bf16, tag="hT")
                    for fl in range(JH):
                        ffc = half * JH + fl
                        w1t = w1pool.tile([P, JD * P], bf16, tag="w1c")
                        nc.sync.dma_start(w1t[:], w1_d[l, ffc])
                        w1sb = w1t.rearrange("p (k m) -> p k m", m=P)
                        ph_ = pproj.tile([P, T], f32, tag="proj")
                        for kc in range(JD):
                            nc.tensor.matmul(
                                ph_[:], lhsT=w1sb[:, kc, :], rhs=ln2.yb[:, kc, :],
                                start=(kc == 0), stop=False)
                        nc.tensor.matmul(
                            ph_[:],
                            lhsT=wsf_sb[0:1, ffc * P:(ffc + 1) * P],
                            rhs=ln2.mnegb[:],
                            start=False, stop=True)
                        if half == 0 and fl == 1:
                            ln2.emit_bcasts()
                        # rstd commutes through relu and the FF contraction;
                        # applied at the FFN2 evacuation
                        nc.scalar.activation(out=hT[:, fl, :], in_=ph_[:],
                                             func=AF.Relu)
                    if half == 0:
                        ln2.emit_tail()  # xT = x2 for the FFN residual
                    for jd in range(JD):
                        w2t = w2pool.tile([P, JH * P], bf16, tag="w2c")
                        nc.sync.dma_start(
                            w2t[:], w2_d[l, jd, :, half * JH * P:(half + 1) * JH * P])
                        w2sb = w2t.rearrange("p (f m) -> p f m", m=P)
                        py = pproj.tile([P, T], f32, tag="proj")
                        for fc in range(JH):
                            nc.tensor.matmul(
                                py[:], lhsT=w2sb[:, fc, :], rhs=hT[:, fc, :],
                                start=(fc == 0), stop=(fc == JH - 1))
                        if half == 0:
                            nc.vector.tensor_tensor(out=y[:, jd, :], in0=py[:],
                                                    in1=ln2.a_sb[:], op=OP.mult)
                            nc.vector.tensor_tensor(out=y[:, jd, :], in0=y[:, jd, :],
                                                    in1=xT[:, jd, :], op=OP.add)
                        else:
                            tmp = spool.tile([P, T], f32, tag="tmp")
                            nc.vector.tensor_tensor(out=tmp[:], in0=py[:],
                                                    in1=ln2.a_sb[:], op=OP.mult)
                            nc.vector.tensor_tensor(out=y[:, jd, :], in0=y[:, jd, :],
                                                    in1=tmp[:], op=OP.add)
                            ln3.chunk(jd)
                ln3.finish_rows()
                ln_prev = ln3

            ln_prev.emit_tail()

        if loop_n > 1:
            with tc.For_i(0, loop_n, 1):
                layers_body()
        else:
            layers_body()
        nc.sync.dma_start(out_d[:], xT_f[:])


def _featmajor(a2d):
    """[T, D] -> [P, D//P, T] feature-major layout."""
    d = a2d.shape[1]
    return np.ascontiguousarray(
        a2d.T.reshape(d // P, P, a2d.shape[0]).transpose(1, 0, 2))


def _prep_weights(sa_w, ca_w, w1, w2):
    out = {}
    for pref, w in (("sa", sa_w), ("ca", ca_w)):
        for i, part in enumerate("qkvo"):
            # natural [Din, Dout] -> [L, 2, P, JD*(D//2)] (output-col halves)
            arr = np.ascontiguousarray(
                w[:, i].reshape(L, JD, P, 2, D // 2).transpose(0, 3, 2, 1, 4)
            ).astype(BF16)
            out[pref + part] = arr.reshape(L, 2, P, JD * (D // 2))
    # w1 [L, Din, FF] -> [L, JF, P, JD*P]: w1c[l, ffc, p, kc*P+m] = w1[l, kc*P+p, ffc*P+m]
    a = w1.reshape(L, JD, P, JF, P)                      # [l, kc, p, ffc, m]
    out["w1"] = np.ascontiguousarray(
        a.transpose(0, 3, 2, 1, 4)).astype(BF16).reshape(L, JF, P, JD * P)
    # w2 [L, FF, D] -> [L, JD, P, JF*P]: w2c[l, jd, p, ffc*P+m] = w2[l, ffc*P+p, jd*P+m]
    a = w2.reshape(L, JF, P, JD, P)                      # [l, ffc, p, jd, m]
    out["w2"] = np.ascontiguousarray(
        a.transpose(0, 3, 2, 1, 4)).astype(BF16).reshape(L, JD, P, JF * P)
    # column sums (over Din) of the bf16-cast weights, for the -mean seeds
    ws = np.zeros((L, 1, 4 * D), np.float32)
    ws[:, 0, OFF_SAQ:OFF_SAQ + D] = sa_w[:, 0].astype(BF16).astype(np.float32).sum(1)
    ws[:, 0, OFF_SAK:OFF_SAK + D] = sa_w[:, 1].astype(BF16).astype(np.float32).sum(1)
    ws[:, 0, OFF_CAQ:OFF_CAQ + D] = ca_w[:, 0].astype(BF16).astype(np.float32).sum(1)
    ws[:, 0, OFF_SAV:OFF_SAV + D] = sa_w[:, 2].astype(BF16).astype(np.float32).sum(1)
    out["wsums"] = ws.astype(BF16)
    out["wsumf"] = w1.astype(BF16).astype(np.float32).sum(1)[:, None, :].astype(BF16)
    return out


def _make_in_maps(trg, enc, mask, wmaps):
    in_maps = []
    for b in range(NCORES):
        m = dict(wmaps)
        xt = _featmajor(trg[b]).reshape(P, JD * T)
        m["xT"] = xt
        m["xTb"] = xt.astype(BF16)
        m["encT"] = _featmajor(enc[b]).astype(BF16).reshape(P, JD * T)
        m01 = (mask[b] != 0).astype(np.float32)
        m["mask01"] = _featmajor(m01).astype(BF16).reshape(P, TC * T)
        in_maps.append(m)
    return in_maps


def kernel(trg, enc, mask, sa_w, sa_b, ca_w, ca_b, ln_g, ln_b, w1, b1, w2, b2,
           _results_hook=None):
    trg = np.asarray(trg, np.float32)
    enc = np.asarray(enc, np.float32)
    mask = np.asarray(mask)
    sa_w = np.asarray(sa_w, np.float32)
    ca_w = np.asarray(ca_w, np.float32)
    w1 = np.asarray(w1, np.float32)
    w2 = np.asarray(w2, np.float32)
    # this kernel folds trivial affine params (the reference initializes biases
    # to zero and layernorm gains to one); verify that assumption holds
    for nm, v in (("sa_b", sa_b), ("ca_b", ca_b), ("ln_b", ln_b), ("b1", b1),
                  ("b2", b2)):
        assert not np.any(np.asarray(v)), f"{nm} nonzero: not supported"
    assert np.all(np.asarray(ln_g) == 1.0), "ln_g != 1 not supported"

    if "nc" not in _CACHE:
        _CACHE["nc"] = _build_program()
    nc = _CACHE["nc"]

    wmaps = _prep_weights(sa_w, ca_w, w1, w2)
    in_maps = _make_in_maps(trg, enc, mask, wmaps)

    from concourse import bass_utils
    kwargs = {}
    if TRACE_TMPDIR is not None:
        kwargs.update(trace=True, tmpdir=TRACE_TMPDIR)
    res = bass_utils.run_bass_kernel_spmd(nc, in_maps,
                                          core_ids=list(range(NCORES)), **kwargs)
    if _results_hook is not None:
        _results_hook(res)

    out = np.empty((NCORES, T, D), np.float32)
    for b in range(NCORES):
        oT = np.asarray(res.results[b]["outT"]).reshape(P, JD, T)
        out[b] = oT.transpose(1, 0, 2).reshape(D, T).T
    return out



# revision 19
# speedup vs baseline: 1.0082x; 1.0082x over previous
"""Trainium2 Bass kernel for a 6-layer transformer decoder (D=1024, H=16, FF=4096).

Sharding: data-parallel over batch — each of the 8 NeuronCores processes one
batch element end-to-end (no collectives).

On-chip layout: activations are kept feature-major ("xT": [D, T] stored as
[128 partitions, D/128, T]) so every linear layer is a natural
`out = W.T @ xT` PE matmul (lhsT = natural-layout weight chunks), layernorm
statistics are computed with ones-vector matmuls on the PE, and per-token
scalars are broadcast across partitions with K=1 outer-product matmuls.
Attention scores are computed transposed (S^T = K Q^T per head) so softmax
normalization lands on the free axis and attn@V consumes the probabilities
directly, with a ones-column appended to V to produce the softmax denominator
in the same accumulation.

Layernorm is algebraically commuted past the following projections so the PE
never waits for the normalization statistics: the projections run on the
*pre-norm* activations, the mean-centering enters each matmul accumulation as
a rank-1 seed (host-precomputed weight column sums x (-mean), one K=1 matmul
appended to each accumulation group), and the 1/std scale is applied during
psum evacuation (for the FFN it commutes through ReLU and the second matmul
and is applied at the FFN2 evacuation).

Matmul operands are bf16 (fp32 PSUM accumulation); the residual stream and
softmax/layernorm statistics stay fp32.
"""
import numpy as np
import ml_dtypes

BF16 = ml_dtypes.bfloat16

L, D, H, FF = 6, 1024, 16, 4096
DH = D // H          # 64
T = 512              # sequence length (both q and kv)
P = 128
JD = D // P          # 8 feature chunks
JF = FF // P         # 32 ff chunks
JH = JF // 2         # ff chunks per half
TC = T // P          # 4 token chunks
NCORES = 8
EPS = 1e-12
VW = DH + 1          # v columns per head incl. ones column (65)
WS = 4 * D + FF      # wsum row length (saq | sak | caq | sav | w1)
OFF_SAQ, OFF_SAK, OFF_CAQ, OFF_SAV, OFF_W1 = 0, D, 2 * D, 3 * D, 4 * D

_CACHE = {}
TRACE_TMPDIR = None  # test harness may set this to capture a profile


def _build_program(n_layers=L, loop_n=1):
    import concourse.bass as bass
    import concourse.mybir as mybir
    import concourse.tile as tile
    from concourse import bacc

    f32 = mybir.dt.float32
    bf16 = mybir.dt.bfloat16
    AF = mybir.ActivationFunctionType
    OP = mybir.AluOpType

    class _OneActTableBacc(bacc.Bacc):
        """This kernel only uses exp/ln/relu/square/copy, all of which live in
        the `natural_log_exp_and_others` activation-table set.  The stock
        first-fit table choice pairs `ln` with a set that lacks `exp`, causing
        a table reload on every layernorm; presenting only the one covering
        set (other slots empty, list order preserved so set ids still match
        act_info.json) collapses all reloads into a single load."""

        def insert_act_table_loads(self):
            from concourse.hw_specs import get_activation_tables
            import bass_rust as _bass_rust
            has_activation = any(
                isinstance(i, mybir.InstActivation)
                for b in self.main_func.blocks
                for i in b.instructions
            )
            if not has_activation:
                return
            tables = list(get_activation_tables(self.m.arch).items())
            keep = "natural_log_exp_and_others"
            assert any(nm == keep for nm, _ in tables)
            tables = [(nm, (fns if nm == keep else set()))
                      for nm, fns in tables]
            _bass_rust.insert_act_table_loads(self, tables)

    nc = _OneActTableBacc(
        "TRN2",
        target_bir_lowering=False,
        debug=False,
        enable_asserts=False,
        num_devices=NCORES,
    )

    # ---- DRAM tensors (per-core shapes) ----
    # inner dims flattened so each DMA is one contiguous run per partition
    xT_d = nc.dram_tensor("xT", [P, JD * T], f32, kind="ExternalInput").ap()
    xTb_d = nc.dram_tensor("xTb", [P, JD * T], bf16, kind="ExternalInput").ap()
    encT_d = nc.dram_tensor("encT", [P, JD * T], bf16, kind="ExternalInput").ap()
    mask_d = nc.dram_tensor("mask01", [P, TC * T], bf16, kind="ExternalInput").ap()
    wnames = ["saq", "sak", "sav", "sao", "caq", "cak", "cav", "cao"]
    wd = {
        n: nc.dram_tensor(n, [L, 2, P, JD * (D // 2)], bf16,
                          kind="ExternalInput").ap()
        for n in wnames
    }
    w1_d = nc.dram_tensor("w1", [L, JF, P, JD * P], bf16, kind="ExternalInput").ap()
    w2_d = nc.dram_tensor("w2", [L, JD, P, JF * P], bf16, kind="ExternalInput").ap()
    ws_d = nc.dram_tensor("wsums", [L, 1, 4 * D], bf16, kind="ExternalInput").ap()
    wsf_d = nc.dram_tensor("wsumf", [L, 1, FF], bf16, kind="ExternalInput").ap()
    out_d = nc.dram_tensor("outT", [P, JD * T], f32, kind="ExternalOutput").ap()

    with tile.TileContext(nc) as tc:
        _emit(tc, nc, bass, mybir, tile, f32, bf16, AF, OP, xT_d, xTb_d,
              encT_d, mask_d, wd, w1_d, w2_d, ws_d, wsf_d, out_d, n_layers,
              loop_n)
    nc.compile()
    return nc


def _emit(tc, nc, bass, mybir, tile, f32, bf16, AF, OP, xT_d, xTb_d,
          encT_d, mask_d, wd, w1_d, w2_d, ws_d, wsf_d, out_d, n_layers=L,
          loop_n=1):
    from contextlib import ExitStack
    ctx = ExitStack()
    with ctx:
        persist = ctx.enter_context(tc.tile_pool(name="persist", bufs=1))
        wpool = ctx.enter_context(tc.tile_pool(name="wpool", bufs=4))
        w1pool = ctx.enter_context(tc.tile_pool(name="w1pool", bufs=4))
        w2pool = ctx.enter_context(tc.tile_pool(name="w2pool", bufs=2))
        wspool = ctx.enter_context(tc.tile_pool(name="wspool", bufs=1))
        apool = ctx.enter_context(tc.tile_pool(name="apool", bufs=1))
        lnpool = ctx.enter_context(tc.tile_pool(name="lnpool", bufs=1))
        spool = ctx.enter_context(tc.tile_pool(name="spool", bufs=2))
        abpool = ctx.enter_context(tc.tile_pool(name="abpool", bufs=4))
        probpool = ctx.enter_context(tc.tile_pool(name="probpool", bufs=2))
        rows = ctx.enter_context(tc.tile_pool(name="rows", bufs=2))
        rows1 = ctx.enter_context(tc.tile_pool(name="rows1", bufs=1))
        hpool = ctx.enter_context(tc.tile_pool(name="hpool", bufs=1))
        # PSUM pools: 8 banks total
        pproj = ctx.enter_context(tc.tile_pool(name="pproj", bufs=2, space="PSUM"))
        psc = ctx.enter_context(tc.tile_pool(name="psc", bufs=2, space="PSUM"))
        pao = ctx.enter_context(tc.tile_pool(name="pao", bufs=2, space="PSUM"))

        # ---- persistent tiles ----
        xT_f = persist.tile([P, JD * T], f32, tag="xT")     # residual stream
        encTb_f = persist.tile([P, JD * T], bf16, tag="encTb")
        mask_f = persist.tile([P, TC * T], bf16, tag="mask01")
        xT = xT_f.rearrange("p (j t) -> p j t", t=T)
        encTb = encTb_f.rearrange("p (j t) -> p j t", t=T)
        mask01 = mask_f.rearrange("p (c t) -> p c t", t=T)
        # layer-0's bf16 input x lives in the (single) yb slot until the
        # first layernorm claims it
        xTb_t = lnpool.tile([P, JD, T], bf16, tag="yb")
        xTb = xTb_t[:]
        ones_b = persist.tile([P, P], bf16, tag="ones_b")
        ones_f = persist.tile([1, P], f32, tag="ones_f")
        eps_c = persist.tile([1, 1], f32, tag="eps_c")
        y_f = persist.tile([P, JD * T], f32, tag="y")
        y = y_f.rearrange("p (j t) -> p j t", t=T)

        # xTb feeds the very first matmuls — keep it alone on the sync queue
        # so the first weight loads queue right behind it; the rest goes via
        # the gpsimd DMA queue in parallel
        nc.sync.dma_start(xTb_t[:], xTb_d.rearrange("p (j t) -> p j t", t=T))
        nc.gpsimd.dma_start(xT_f[:], xT_d[:])
        nc.gpsimd.dma_start(encTb_f[:], encT_d[:])
        nc.gpsimd.dma_start(mask_f[:], mask_d[:])
        nc.vector.memset(eps_c[:], EPS)
        nc.vector.memset(ones_b[:], 1.0)
        nc.vector.memset(ones_f[:], 1.0)

        def load_wmat(name, l, half):
            """Half of a [Din, Dout] mat: output columns half*512..+512,
            viewed [P, JD, 512]."""
            wt = wpool.tile([P, JD * (D // 2)], bf16, tag="wmat")
            nc.sync.dma_start(wt[:], wd[name][l, half])
            return wt.rearrange("p (k n) -> p k n", n=D // 2)

        # ---------------- layernorm (commuted) ----------------
        class LN:
            """Stats accumulate while the producing projection evacuates;
            centering/scale are applied by the *consuming* projections."""

            def __init__(self, need_cols=False):
                self.need_cols = need_cols

            def start(self):
                self.yb = lnpool.tile([P, JD, T], bf16, tag="yb")
                self.pm = psc.tile([1, T], f32, tag="sc")
                self.pss = psc.tile([1, T], f32, tag="sc")
                self.bcasts_done = False

            def chunk(self, j):
                """Call after y[:, j, :] is written."""
                nc.scalar.copy(out=self.yb[:, j, :], in_=y[:, j, :])
                sq = spool.tile([P, T], bf16, tag="ysqc")
                nc.scalar.activation(out=sq[:], in_=y[:, j, :], func=AF.Square)
                nc.tensor.matmul(self.pm[:], lhsT=ones_b[:, 0:1],
                                 rhs=self.yb[:, j, :],
                                 start=(j == 0), stop=(j == JD - 1))
                nc.tensor.matmul(self.pss[:], lhsT=ones_b[:, 0:1], rhs=sq[:],
                                 start=(j == 0), stop=(j == JD - 1))

            def finish_rows(self):
                # -mean in bf16: doubles as the rank-1 seed operand, and the
                # bf16 rounding is negligible against the bf16 matmul inputs
                m_neg = rows.tile([1, T], bf16, tag="mrow")
                with nc.allow_low_precision(reason="mean in bf16"):
                    nc.vector.tensor_scalar_mul(m_neg[:], self.pm[:], -1.0 / D)
                m2 = rows1.tile([1, T], f32, tag="lr1")
                nc.vector.tensor_tensor(out=m2[:], in0=m_neg[:], in1=m_neg[:],
                                        op=OP.mult)
                var = rows1.tile([1, T], f32, tag="lr2")
                nc.vector.scalar_tensor_tensor(
                    out=var[:], in0=self.pss[:], scalar=1.0 / D, in1=m2[:],
                    op0=OP.mult, op1=OP.subtract)
                lnv = rows1.tile([1, T], f32, tag="lr1")
                nc.scalar.activation(out=lnv[:], in_=var[:], func=AF.Ln,
                                     bias=eps_c[:])
                rstd = rows1.tile([1, T], f32, tag="rstd")
                nc.scalar.activation(out=rstd[:], in_=lnv[:], func=AF.Exp,
                                     scale=-0.5)
                c_row = rows1.tile([1, T], f32, tag="crow")
                nc.vector.tensor_tensor(out=c_row[:], in0=m_neg[:],
                                        in1=rstd[:], op=OP.mult)
                self.mnegb, self.rstd, self.c_row = m_neg, rstd, c_row
                if self.need_cols:
                    # token-indexed columns of rstd and 1/rstd (for scaling
                    # the next self-attention's probabilities / V ones col):
                    # tiny K=1,N=1 matmuls transpose the rows
                    std = rows1.tile([1, T], f32, tag="lr2")
                    nc.scalar.activation(out=std[:], in_=lnv[:], func=AF.Exp,
                                         scale=0.5)
                    pcol = psc.tile([P, 2 * TC], f32, tag="sc")
                    for c in range(TC):
                        nc.tensor.matmul(
                            pcol[:, c:c + 1],
                            lhsT=self.rstd[0:1, c * P:(c + 1) * P],
                            rhs=ones_f[0:1, 0:1],
                            start=True, stop=True)
                        nc.tensor.matmul(
                            pcol[:, TC + c:TC + c + 1],
                            lhsT=std[0:1, c * P:(c + 1) * P],
                            rhs=ones_f[0:1, 0:1],
                            start=True, stop=True)
                    cols = rows.tile([P, 2 * TC], f32, tag="cols")
                    nc.vector.tensor_copy(out=cols[:], in_=pcol[:])
                    self.a_col = cols[:, 0:TC]
                    self.inva_col = cols[:, TC:2 * TC]

            def emit_bcasts(self):
                if self.bcasts_done:
                    return
                self.bcasts_done = True
                pa = psc.tile([P, T], f32, tag="sc")
                nc.tensor.matmul(pa[:], lhsT=ones_f[:, :], rhs=self.rstd[:],
                                 start=True, stop=True)
                self.a_sb = abpool.tile([P, T], f32, tag="ab")
                nc.scalar.copy(out=self.a_sb[:], in_=pa[:])
                pc = psc.tile([P, T], f32, tag="sc")
                nc.tensor.matmul(pc[:], lhsT=ones_f[:, :], rhs=self.c_row[:],
                                 start=True, stop=True)
                self.c_sb = abpool.tile([P, T], f32, tag="ab")
                nc.scalar.copy(out=self.c_sb[:], in_=pc[:])

            def emit_tail(self):
                """Materialize the normalized x (fp32, for the residual adds),
                chunked so it pipelines with surrounding DVE work."""
                self.emit_bcasts()
                for j in range(JD):
                    nc.vector.tensor_tensor(out=y[:, j, :], in0=y[:, j, :],
                                            in1=self.a_sb[:], op=OP.mult)
                    nc.vector.tensor_tensor(out=xT[:, j, :], in0=y[:, j, :],
                                            in1=self.c_sb[:], op=OP.add)

        def proj_seeded(w_sb, ln, ws_sb, ws_off, dst):
            """Feature-major projection reading pre-norm yb, centered by the
            wsum x (-mean) rank-1 seed, scaled by rstd at evacuation."""
            for j in range(JD):
                ps = pproj.tile([P, T], f32, tag="proj")
                for kc in range(JD):
                    nc.tensor.matmul(
                        ps[:],
                        lhsT=w_sb[:, kc, j * P:(j + 1) * P],
                        rhs=ln.yb[:, kc, :],
                        start=(kc == 0), stop=False)
                nc.tensor.matmul(
                    ps[:],
                    lhsT=ws_sb[0:1, ws_off + j * P:ws_off + (j + 1) * P],
                    rhs=ln.mnegb[:],
                    start=False, stop=True)
                ln.emit_bcasts()
                nc.vector.tensor_tensor(out=dst[:, j, :], in0=ps[:],
                                        in1=ln.a_sb[:], op=OP.mult)

        def proj_plain(w_sb, src_b, dst):
            for j in range(JD):
                ps = pproj.tile([P, T], f32, tag="proj")
                for kc in range(JD):
                    nc.tensor.matmul(
                        ps[:],
                        lhsT=w_sb[:, kc, j * P:(j + 1) * P],
                        rhs=src_b[:, kc, :],
                        start=(kc == 0), stop=(kc == JD - 1))
                nc.scalar.copy(out=dst[:, j, :], in_=ps[:])

        def attn_block(wnames, l, ws_sb, ln_src, kv_b, mask, ln_next,
                       tail_ln=None):
            """One attention block with interleaved emission: q/k/v projection
            groups are threaded between the per-head softmax chains so the PE
            stays dense while exp/normalize resolve on ACT/DVE.

            ln_src: LN whose (pre-norm) activations feed q [and k,v for
            self-attention]; None -> plain reads of xTb (layer-0 self-attn).
            kv_b: bf16 feature-major source for k/v when they are NOT fed by
            ln_src (cross-attention: encTb; layer-0 self-attn: xTb).
            tail_ln: LN whose fp32 x tail must materialize before the output
            projection's residual add (emitted late, after the head chains).
            """
            wq_name, wk_name, wv_name, wo_name = wnames
            self_attn = wv_name == "sav"
            seeded = ln_src is not None
            sa_seeded = seeded and self_attn

            qTb = apool.tile([P, JD, T], bf16, tag="qTb")
            kTb = apool.tile([P, JD, T], bf16, tag="kTb")
            vtok = apool.tile([P, TC, H * VW], bf16, tag="vtok")
            vt4 = vtok.rearrange("p tc (h w) -> p tc h w", w=VW)
            wtiles = {}

            def wmat(name, half):
                if (name, half) not in wtiles:
                    wtiles[(name, half)] = load_wmat(name, l, half)
                return wtiles[(name, half)]

            def qk_group(j):
                half, jj = j // 4, j % 4
                for nm, dst, off in ((wq_name, qTb, OFF_SAQ if self_attn
                                      else OFF_CAQ),
                                     (wk_name, kTb, OFF_SAK)):
                    w_sb = wmat(nm, half)
                    if seeded and (self_attn or dst is qTb):
                        ln = ln_src
                        ps = pproj.tile([P, T], f32, tag="proj")
                        for kc in range(JD):
                            nc.tensor.matmul(
                                ps[:], lhsT=w_sb[:, kc, jj * P:(jj + 1) * P],
                                rhs=ln.yb[:, kc, :],
                                start=(kc == 0), stop=False)
                        nc.tensor.matmul(
                            ps[:],
                            lhsT=ws_sb[0:1, off + j * P:off + (j + 1) * P],
                            rhs=ln.mnegb[:], start=False, stop=True)
                        ln.emit_bcasts()
                        nc.vector.tensor_tensor(out=dst[:, j, :], in0=ps[:],
                                                in1=ln.a_sb[:], op=OP.mult)
                    else:
                        ps = pproj.tile([P, T], f32, tag="proj")
                        for kc in range(JD):
                            nc.tensor.matmul(
                                ps[:], lhsT=w_sb[:, kc, jj * P:(jj + 1) * P],
                                rhs=kv_b[:, kc, :],
                                start=(kc == 0), stop=(kc == JD - 1))
                        nc.scalar.copy(out=dst[:, j, :], in_=ps[:])

            def v_group(tc4, jh):
                wv = wmat(wv_name, jh)
                ps = pproj.tile([P, T], f32, tag="proj")
                if sa_seeded:
                    for kc in range(JD):
                        nc.tensor.matmul(
                            ps[:],
                            lhsT=ln_src.yb[:, kc, tc4 * P:(tc4 + 1) * P],
                            rhs=wv[:, kc, :],
                            start=(kc == 0), stop=False)
                    # centering seed: outer(-mean[tc4 chunk], wsum_v[jh half])
                    nc.tensor.matmul(
                        ps[:],
                        lhsT=ln_src.mnegb[0:1, tc4 * P:(tc4 + 1) * P],
                        rhs=ws_sb[0:1, OFF_SAV + jh * 512:
                                  OFF_SAV + (jh + 1) * 512],
                        start=False, stop=True)
                else:
                    for kc in range(JD):
                        nc.tensor.matmul(
                            ps[:],
                            lhsT=kv_b[:, kc, tc4 * P:(tc4 + 1) * P],
                            rhs=wv[:, kc, :],
                            start=(kc == 0), stop=(kc == JD - 1))
                nc.scalar.copy(
                    out=vt4[:, tc4, jh * 8:(jh + 1) * 8, 0:DH],
                    in_=ps.rearrange("p (h d) -> p h d", d=DH))

            # softmax denominator column of V: 1/rstd when the probabilities
            # carry the rstd scale, else plain ones
            if sa_seeded:
                nc.vector.tensor_copy(
                    out=vt4[:, :, :, DH:DH + 1],
                    in_=ln_src.inva_col[:, :, None, None].to_broadcast(
                        [P, TC, H, 1]))
            else:
                nc.vector.memset(vt4[:, :, :, DH:DH + 1], 1.0)

            def head_pair(hp):
                """Heads 2hp (partitions 0:64) and 2hp+1 (64:128) share the
                q/k chunk jh=hp; their score matmuls land in one two-bank
                psum tile via concurrent PE row-groups, so exp and
                mask/scale run once per ktc at double width."""
                jh = hp
                # 2-deep ktc ring of pair-probs (both heads side by side)
                probs = probpool.tile([P, 2, 2, T], bf16, tag="probs")
                po = [pao.tile([P, T], f32, tag="ao", name=f"po{i}")
                      for i in range(2)]
                for ktc in range(TC):
                    ss = psc.tile([P, 2, T], f32, tag="sc")
                    for hh in range(2):
                        ph = hh * 64
                        nc.tensor.matmul(
                            ss[:, hh, :],
                            lhsT=kTb[ph:ph + 64, jh, ktc * P:(ktc + 1) * P],
                            rhs=qTb[ph:ph + 64, jh, :],
                            start=True, stop=True)
                    pr = probs[:, ktc % 2, :, :]
                    nc.scalar.activation(
                        out=pr, in_=ss[:], func=AF.Exp, scale=0.125)
                    if sa_seeded:
                        # fold the V-side rstd (token-indexed, so per-
                        # partition here) and the mask into one pass
                        if mask is not None:
                            nc.vector.scalar_tensor_tensor(
                                out=pr, in0=pr,
                                scalar=ln_src.a_col[:, ktc:ktc + 1],
                                in1=mask[:, ktc, None, :].to_broadcast(
                                    [P, 2, T]),
                                op0=OP.mult, op1=OP.mult)
                        else:
                            nc.vector.tensor_scalar_mul(
                                pr, pr, ln_src.a_col[:, ktc:ktc + 1])
                    elif mask is not None:
                        nc.vector.tensor_tensor(
                            out=pr, in0=pr,
                            in1=mask[:, ktc, None, :].to_broadcast([P, 2, T]),
                            op=OP.mult)
                    for hh in range(2):
                        nc.tensor.matmul(
                            po[hh][0:VW, :],
                            lhsT=vt4[:, ktc, 2 * hp + hh, :],
                            rhs=probs[:, ktc % 2, hh, :],
                            start=(ktc == 0), stop=(ktc == TC - 1))
                for hh in range(2):
                    ph = hh * 64
                    r_row = rows.tile([1, T], bf16, tag="rrow")
                    with nc.allow_low_precision(reason="softmax denom bf16"):
                        nc.vector.reciprocal(out=r_row[:],
                                             in_=po[hh][DH:DH + 1, :])
                    pb = psc.tile([P, 2, T], f32, tag="sc")
                    nc.tensor.matmul(
                        pb[0:DH, 0, :], lhsT=ones_b[0:1, 0:DH], rhs=r_row[:],
                        start=True, stop=True)
                    bb = spool.tile([DH, T], bf16, tag="bcast_sb")
                    nc.vector.tensor_copy(out=bb[:], in_=pb[0:DH, 0, :])
                    nc.vector.tensor_tensor(
                        out=attnTb[ph:ph + 64, jh, :], in0=po[hh][0:DH, :],
                        in1=bb[:], op=OP.mult)

            attnTb = apool.tile([P, JD, T], bf16, tag="attnTb")
            # interleaved emission: enough q/k/v groups up front for the
            # first head pairs, the rest threaded between head chains
            qk_group(0)
            qk_group(1)
            for tc4 in range(TC):
                v_group(tc4, 0)
            extra = {0: [("qk", 2)], 1: [("qk", 3), ("v", 0), ("v", 1)],
                     2: [("qk", 4), ("v", 2), ("v", 3)], 3: [("qk", 5)],
                     4: [("qk", 6)], 5: [("qk", 7)]}
            for hp in range(JD):
                head_pair(hp)
                for kind, idx in extra.get(hp, []):
                    if kind == "qk":
                        qk_group(idx)
                    else:
                        v_group(idx, 1)
            if tail_ln is not None:
                tail_ln.emit_tail()
            ln_next.start()
            for hp in range(JD):
                # output projection (+ residual + LN stats chunk)
                wo = wmat(wo_name, hp // 4)
                ps = pproj.tile([P, T], f32, tag="proj")
                for kc in range(JD):
                    nc.tensor.matmul(
                        ps[:],
                        lhsT=wo[:, kc, (hp % 4) * P:(hp % 4 + 1) * P],
                        rhs=attnTb[:, kc, :],
                        start=(kc == 0), stop=(kc == JD - 1))
                nc.vector.tensor_tensor(out=y[:, hp, :], in0=ps[:],
                                        in1=xT[:, hp, :], op=OP.add)
                ln_next.chunk(hp)
            ln_next.finish_rows()

        def layers_body():
            ln_prev = None  # LN3 of the previous layer (None for layer 0)
            for l in range(n_layers):
                ws_sb = wspool.tile([1, 4 * D], bf16, tag="wsums")
                nc.sync.dma_start(ws_sb[:], ws_d[l])

                # --- self-attention block ---
                # (in the For_i timing build the layer-0 input tile's slot is
                # recycled by later layernorms, so point it at a persistent
                # tile instead; timing-equivalent, numerics unused there)
                x0 = encTb if loop_n > 1 else xTb
                ln1 = LN()
                attn_block(("saq", "sak", "sav", "sao"), l, ws_sb, ln_prev,
                           x0, mask01, ln1, tail_ln=ln_prev)

                # --- cross-attention block ---
                ln2 = LN()
                attn_block(("caq", "cak", "cav", "cao"), l, ws_sb, ln1,
                           encTb, None, ln2, tail_ln=ln1)

                # --- FFN ---
                wsf_sb = wspool.tile([1, FF], bf16, tag="wsumf")
                nc.sync.dma_start(wsf_sb[:], wsf_d[l])
                ln3 = LN(need_cols=True)
                ln3.start()
                for half in range(2):
                    hT = hpool.tile([P, JH, T], bf16, tag="hT")
                    for fl in range(JH):
                        ffc = half * JH + fl
                        w1t = w1pool.tile([P, JD * P], bf16, tag="w1c")
                        nc.sync.dma_start(w1t[:], w1_d[l, ffc])
                        w1sb = w1t.rearrange("p (k m) -> p k m", m=P)
                        ph_ = pproj.tile([P, T], f32, tag="proj")
                        for kc in range(JD):
                            nc.tensor.matmul(
                                ph_[:], lhsT=w1sb[:, kc, :], rhs=ln2.yb[:, kc, :],
                                start=(kc == 0), stop=False)
                        nc.tensor.matmul(
                            ph_[:],
                            lhsT=wsf_sb[0:1, ffc * P:(ffc + 1) * P],
                            rhs=ln2.mnegb[:],
                            start=False, stop=True)
                        if half == 0 and fl == 1:
                            ln2.emit_bcasts()
                        # rstd commutes through relu and the FF contraction;
                        # applied at the FFN2 evacuation
                        nc.scalar.activation(out=hT[:, fl, :], in_=ph_[:],
                                             func=AF.Relu)
                    if half == 0:
                        ln2.emit_tail()  # xT = x2 for the FFN residual
                    for jd in range(JD):
                        w2t = w2pool.tile([P, JH * P], bf16, tag="w2c")
                        nc.sync.dma_start(
                            w2t[:], w2_d[l, jd, :, half * JH * P:(half + 1) * JH * P])
                        w2sb = w2t.rearrange("p (f m) -> p f m", m=P)
                        py = pproj.tile([P, T], f32, tag="proj")
                        for fc in range(JH):
                            nc.tensor.matmul(
                                py[:], lhsT=w2sb[:, fc, :], rhs=hT[:, fc, :],
                                start=(fc == 0), stop=(fc == JH - 1))
                        if half == 0:
                            nc.vector.tensor_tensor(out=y[:, jd, :], in0=py[:],
                                                    in1=ln2.a_sb[:], op=OP.mult)
                            nc.vector.tensor_tensor(out=y[:, jd, :], in0=y[:, jd, :],
                                                    in1=xT[:, jd, :], op=OP.add)
                        else:
                            tmp = spool.tile([P, T], f32, tag="tmp")
                            nc.vector.tensor_tensor(out=tmp[:], in0=py[:],
                                                    in1=ln2.a_sb[:], op=OP.mult)
                            nc.vector.tensor_tensor(out=y[:, jd, :], in0=y[:, jd, :],
                                                    in1=tmp[:], op=OP.add)
                            ln3.chunk(jd)
                ln3.finish_rows()
                ln_prev = ln3

            ln_prev.emit_tail()

        if loop_n > 1:
            with tc.For_i(0, loop_n, 1):
                layers_body()
        else:
            layers_body()
        nc.sync.dma_start(out_d[:], xT_f[:])


def _featmajor(a2d):
    """[T, D] -> [P, D//P, T] feature-major layout."""
    d = a2d.shape[1]
    return np.ascontiguousarray(
        a2d.T.reshape(d // P, P, a2d.shape[0]).transpose(1, 0, 2))


def _prep_weights(sa_w, ca_w, w1, w2):
    out = {}
    for pref, w in (("sa", sa_w), ("ca", ca_w)):
        for i, part in enumerate("qkvo"):
            # natural [Din, Dout] -> [L, 2, P, JD*(D//2)] (output-col halves)
            arr = np.ascontiguousarray(
                w[:, i].reshape(L, JD, P, 2, D // 2).transpose(0, 3, 2, 1, 4)
            ).astype(BF16)
            out[pref + part] = arr.reshape(L, 2, P, JD * (D // 2))
    # w1 [L, Din, FF] -> [L, JF, P, JD*P]: w1c[l, ffc, p, kc*P+m] = w1[l, kc*P+p, ffc*P+m]
    a = w1.reshape(L, JD, P, JF, P)                      # [l, kc, p, ffc, m]
    out["w1"] = np.ascontiguousarray(
        a.transpose(0, 3, 2, 1, 4)).astype(BF16).reshape(L, JF, P, JD * P)
    # w2 [L, FF, D] -> [L, JD, P, JF*P]: w2c[l, jd, p, ffc*P+m] = w2[l, ffc*P+p, jd*P+m]
    a = w2.reshape(L, JF, P, JD, P)                      # [l, ffc, p, jd, m]
    out["w2"] = np.ascontiguousarray(
        a.transpose(0, 3, 2, 1, 4)).astype(BF16).reshape(L, JD, P, JF * P)
    # column sums (over Din) of the bf16-cast weights, for the -mean seeds
    ws = np.zeros((L, 1, 4 * D), np.float32)
    ws[:, 0, OFF_SAQ:OFF_SAQ + D] = sa_w[:, 0].astype(BF16).astype(np.float32).sum(1)
    ws[:, 0, OFF_SAK:OFF_SAK + D] = sa_w[:, 1].astype(BF16).astype(np.float32).sum(1)
    ws[:, 0, OFF_CAQ:OFF_CAQ + D] = ca_w[:, 0].astype(BF16).astype(np.float32).sum(1)
    ws[:, 0, OFF_SAV:OFF_SAV + D] = sa_w[:, 2].astype(BF16).astype(np.float32).sum(1)
    out["wsums"] = ws.astype(BF16)
    out["wsumf"] = w1.astype(BF16).astype(np.float32).sum(1)[:, None, :].astype(BF16)
    return out


def _make_in_maps(trg, enc, mask, wmaps):
    in_maps = []
    for b in range(NCORES):
        m = dict(wmaps)
        xt = _featmajor(trg[b]).reshape(P, JD * T)
        m["xT"] = xt
        m["xTb"] = xt.astype(BF16)
        m["encT"] = _featmajor(enc[b]).astype(BF16).reshape(P, JD * T)
        m01 = (mask[b] != 0).astype(np.float32)
        m["mask01"] = _featmajor(m01).astype(BF16).reshape(P, TC * T)
        in_maps.append(m)
    return in_maps


def kernel(trg, enc, mask, sa_w, sa_b, ca_w, ca_b, ln_g, ln_b, w1, b1, w2, b2,
           _results_hook=None):
    trg = np.asarray(trg, np.float32)
    enc = np.asarray(enc, np.float32)
    mask = np.asarray(mask)
    sa_w = np.asarray(sa_w, np.float32)
    ca_w = np.asarray(ca_w, np.float32)
    w1 = np.asarray(w1, np.float32)
    w2 = np.asarray(w2, np.float32)
    # this kernel folds trivial affine params (the reference initializes biases
    # to zero and layernorm gains to one); verify that assumption holds
    for nm, v in (("sa_b", sa_b), ("ca_b", ca_b), ("ln_b", ln_b), ("b1", b1),
                  ("b2", b2)):
        assert not np.any(np.asarray(v)), f"{nm} nonzero: not supported"
    assert np.all(np.asarray(ln_g) == 1.0), "ln_g != 1 not supported"

    if "nc" not in _CACHE:
        _CACHE["nc"] = _build_program()
    nc = _CACHE["nc"]

    wmaps = _prep_weights(sa_w, ca_w, w1, w2)
    in_maps = _make_in_maps(trg, enc, mask, wmaps)

    from concourse import bass_utils
    kwargs = {}
    if TRACE_TMPDIR is not None:
        kwargs.update(trace=True, tmpdir=TRACE_TMPDIR)
    res = bass_utils.run_bass_kernel_spmd(nc, in_maps,
                                          core_ids=list(range(NCORES)), **kwargs)
    if _results_hook is not None:
        _results_hook(res)

    out = np.empty((NCORES, T, D), np.float32)
    for b in range(NCORES):
        oT = np.asarray(res.results[b]["outT"]).reshape(P, JD, T)
        out[b] = oT.transpose(1, 0, 2).reshape(D, T).T
    return out

